# revision 14
# baseline (speedup 1.0000x reference)
"""Trainium2 Bass kernel for a 1D Kernel Neural Operator (KNO) on a regular grid.

Reference computation (N=2048 nodes, C=32 channels, DEPTH=3):
    fq = gelu([f_x, x] @ lift_W.T + lift_b)
    for i in 0..2:
        skip  = fq @ pw_W[i].T + pw_b[i]
        K_c   = sig2_c * exp(-(x_n - x_q)^2 * a_c),  a_c = 1/(2*ell2_c)
        integ = einsum('cnq,qc->nc', K, fq * w)
        fq    = skip + integ; gelu if i < 2
    out = (gelu(gelu(fq@W1.T+b1)@W2.T+b2)) @ W3.T + b3

Instead of materializing the C x N x N kernels, we use the factorization
exp(-a(x_n-x_q)^2) = e^{-a x_n^2} e^{2 a x_n x_q} e^{-a x_q^2} with the Taylor
expansion e^{2a x_n x_q} = sum_k (2a)^k/k! x_n^k x_q^k (K=32 terms; truncation
error < 1e-4 relative for the a-range here). Each layer's integral is then two
matmuls through the moment basis V[n,k] = x_n^k:
    U       = fq ⊙ (w_q e^{-a_c x_q^2})             [N,C]
    M[k,c]  = sum_q V[q,k] U[q,c]                   [K,C]   (PE, 16 psum-accum)
    Mt      = M ⊙ B,  B[k,c] = (2a_c)^k/k!
    integ   = (s2_c e^{-a_c x_n^2}) ⊙ (VT.T @ Mt)   [N,C]   (PE)

All basis/constant tensors (V, VT, w e^{-a x^2}, s2 e^{-a x^2}, B, block-diag
mixing weights) are pure functions of the non-f_x inputs and are precomputed on
the host; the device program is only the f_x-dependent dependency chain. All
matmul operands are bf16 (1 PE cycle/row vs 4 for fp32; rel-l2 vs the fp32
reference is ~2e-3, tolerance 2e-2); PSUM accumulation stays fp32.

Data layout: the [N,C] state lives channel-transposed in a 4-chunk stack
fqT[32j + c, n'] = fq[512j + n', c], a single [128, 512] SBUF tile. Channel
mixing (pw/proj) is ONE matmul per layer with host-built block-diagonal
[128,128] weights; biases ride the gelu activation's per-partition bias port.
The moment contraction needs q on partitions, so each layer does 4 PE
transposes of fqT back to natural layout. For layers 1-2 the moment matmuls
use a 4x-replicated basis (built once on-device by a stride-0 broadcast DVE
copy) so M lands replicated across partition groups and a single broadcast
multiply against a block-diag-masked B4 builds the [128,128] eval stationary;
layer 0 (whose replicated basis could not arrive in time by DMA) uses the
compact basis and 4 partition-shifted multiplies instead.

Scheduling: constants stream via 9 DMAs spread over the three engine DMA
rings (which run concurrently at ~140GB/s each), FIFO-ordered by need; the
gelu table prefetches during the DMA wait; eval/combine/gelu are split into
256-column halves so PE/DVE/ACT pipeline across halves; the skip matmul runs
in the PE idle slot during the Mt4 build; the skip stages to SBUF bf16 via
the idle ACT engine so the pre-add runs in the DVE 2x 16-bit mode.

Sharding: the whole problem is a dependent chain of small ops, so all 8 cores
run identical replicas (collectives cost more than they save); the output is
taken from core 0.
"""

import numpy as np
import ml_dtypes

import concourse.bass as bass
import concourse.tile as tile
from concourse import bacc, mybir
from concourse.bass_utils import run_bass_kernel_spmd

N = 2048
C = 32
K = 32
DEPTH = 3
NCORES = 8
F32 = mybir.dt.float32
BF16 = mybir.dt.bfloat16
AF = mybir.ActivationFunctionType
ALU = mybir.AluOpType
NPBF16 = ml_dtypes.bfloat16

DID_W = 128    # transpose identity
D0_W = 640     # liftT + liftin (12 partitions)
DT0_W = 512    # G0 = e^{-a0 x^2} natural layout
DX2_W = 512    # x^2 in T layout (Es generation)
DHD_W = 260    # p1W + p2W + sel3
DY1_W = 672    # VTs + B0 + pwW0
DL_W = 768     # per-layer: G_i natural + B4M + pwW  (layers 1, 2)
DVM_W = 512    # Vm' = w x^k
CF_W = 11
H = 256  # column half for the pipelined stages

_CACHE = {}


def build_program(nc):
    did = nc.dram_tensor("did", [128, DID_W], BF16, kind="ExternalInput")
    d0 = nc.dram_tensor("d0", [12, D0_W], BF16, kind="ExternalInput")
    dt0 = nc.dram_tensor("dt0", [128, DT0_W], BF16, kind="ExternalInput")
    dx2 = nc.dram_tensor("dx2", [128, DX2_W], BF16, kind="ExternalInput")
    dhd = nc.dram_tensor("dhd", [128, DHD_W], BF16, kind="ExternalInput")
    dy1 = nc.dram_tensor("dy1", [128, DY1_W], BF16, kind="ExternalInput")
    dl1 = nc.dram_tensor("dl1", [128, DL_W], BF16, kind="ExternalInput")
    dl2 = nc.dram_tensor("dl2", [128, DL_W], BF16, kind="ExternalInput")
    dvm = nc.dram_tensor("dvm", [128, DVM_W], BF16, kind="ExternalInput")
    cf = nc.dram_tensor("cf", [128, CF_W], F32, kind="ExternalInput")
    out_dram = nc.dram_tensor("out", [4, 512], F32, kind="ExternalOutput")

    with tile.TileContext(nc) as tc:
        with (
            tc.tile_pool(name="const", bufs=1) as cp,
            tc.tile_pool(name="work", bufs=3) as wp,
            tc.tile_pool(name="pmix", bufs=2, space="PSUM") as pmix,
            tc.tile_pool(name="ptr", bufs=1, space="PSUM") as ptr,
            tc.tile_pool(name="pmom", bufs=1, space="PSUM") as pmom,
            tc.tile_pool(name="pev", bufs=1, space="PSUM") as pev,
            tc.tile_pool(name="pout", bufs=1, space="PSUM") as pop,
        ):
            # warm-act source and home of the eval stationary
            Mt4 = cp.tile([128, 128], BF16, tag="Mt4")
            nc.vector.memset(Mt4[:], 0.0)

            # Constant streams: three concurrent DMA rings (one per issuing
            # engine), FIFO within a ring, ordered by first use.
            didt = cp.tile([128, DID_W], BF16, tag="didt")
            nc.scalar.dma_start(didt[:], did[:])
            d0t = cp.tile([12, D0_W], BF16, tag="d0t")
            nc.scalar.dma_start(d0t[:], d0[:])
            dt0t = cp.tile([128, DT0_W], BF16, tag="dt0t")
            nc.scalar.dma_start(dt0t[:], dt0[:])
            dx2t = cp.tile([128, DX2_W], BF16, tag="dx2t")
            nc.scalar.dma_start(dx2t[:], dx2[:])

            dy1t = cp.tile([128, DY1_W], BF16, tag="dy1t")
            nc.sync.dma_start(dy1t[:], dy1[:])
            dl1t = cp.tile([128, DL_W], BF16, tag="dl1t")
            nc.sync.dma_start(dl1t[:], dl1[:])
            dl2t = cp.tile([128, DL_W], BF16, tag="dl2t")
            nc.sync.dma_start(dl2t[:], dl2[:])
            dhdt = cp.tile([128, DHD_W], BF16, tag="dhdt")
            nc.sync.dma_start(dhdt[:], dhd[:])

            dvmt = cp.tile([128, DVM_W], BF16, tag="dvmt")
            nc.gpsimd.dma_start(dvmt[:], dvm[:])
            cft = cp.tile([128, CF_W], F32, tag="cft")
            nc.gpsimd.dma_start(cft[:], cf[:])

            # prefetch the gelu activation table during the const DMAs
            warm = wp.tile([1, 8], BF16, tag="warm")
            nc.scalar.activation(warm[:], Mt4[0:1, 0:8], AF.Gelu_apprx_tanh)

            ident = didt[:, 0:128]
            liftT = d0t[0:12, 0:128]
            liftin = d0t[0:12, 128:640]
            tmpN = [dt0t[:, 0:512], dl1t[:, 0:512], dl2t[:, 0:512]]
            VTs = dy1t[:, 0:512]
            B0 = dy1t[0:K, 512:544]
            pwW = [dy1t[:, 544:672], dl1t[:, 640:768], dl2t[:, 640:768]]
            B4M = [None, dl1t[:, 512:640], dl2t[:, 512:640]]
            Vm = dvmt[:, 0:512]
            p1W = dhdt[:, 0:128]
            p2W = dhdt[:, 128:256]
            sel3 = dhdt[:, 256:260]
            pwb = [cft[:, 0:1], cft[:, 1:2]]
            p1b = cft[:, 2:3]
            p2b = cft[:, 3:4]
            b3c = cft[0:4, 4:5]
            nega = [cft[:, 5 + i:6 + i] for i in range(3)]
            lnsg = [cft[:, 8 + i:9 + i] for i in range(3)]

            # Es_i[32j+c, n'] = sig2_c e^{-a_c x_n^2} generated on-device:
            # one Exp activation per layer from the shared x^2 table
            EsT = []
            for i in range(DEPTH):
                e = cp.tile([128, 512], BF16, name=f"Es{i}", tag=f"Es{i}")
                nc.scalar.activation(e[:], dx2t[:], AF.Exp, bias=lnsg[i], scale=nega[i])
                EsT.append(e)

            # ---------------- lift ----------------
            liftp = pmix.tile([128, 512], F32, tag="mix")
            nc.tensor.matmul(liftp[:], liftT, liftin, start=True, stop=True)
            fq = wp.tile([128, 512], BF16, tag="fq")
            for h in range(2):
                nc.scalar.activation(
                    fq[:, H * h:H * (h + 1)], liftp[:, H * h:H * (h + 1)],
                    AF.Gelu_apprx_tanh,
                )

            # 4x-replicated moment basis for layers 1-2, built on-device by a
            # stride-0 broadcast copy while layer 0 runs
            Vm4 = cp.tile([128, 2048], BF16, tag="Vm4")

            # ---------------- KNO layers ----------------
            for i in range(DEPTH):
                # natural layout: trp[p, 128m + 32j + c] = fq[512j+128m+p, c]
                trp = ptr.tile([128, 512], BF16, tag="trp")
                skp = pmix.tile([128, 512], F32, tag="mix")
                U = wp.tile([128, 512], BF16, tag="U")
                skps = wp.tile([128, 512], BF16, tag="skps")
                for m in range(4):
                    nc.tensor.transpose(
                        trp[:, 128 * m:128 * (m + 1)],
                        fq[:, 128 * m:128 * (m + 1)],
                        ident,
                    )
                # U = fq_nat * (w e^{-a x^2}), halves so moments start early
                for h in range(2):
                    nc.vector.tensor_mul(
                        U[:, H * h:H * (h + 1)],
                        trp[:, H * h:H * (h + 1)],
                        tmpN[i][:, H * h:H * (h + 1)],
                    )
                if i == 0:
                    # moments M[k,c] = sum_q V[q,k] U[q,c] (compact basis)
                    Mp = pmom.tile([K, C], F32, tag="Mp")
                    for t in range(16):
                        m, j = divmod(t, 4)
                        nc.tensor.matmul(
                            Mp[:],
                            Vm[:, K * t:K * (t + 1)],
                            U[:, 128 * m + 32 * j:128 * m + 32 * j + 32],
                            start=(t == 0),
                            stop=(t == 15),
                        )
                else:
                    # replicated basis -> M lands on all 4 partition groups
                    Mp4 = pmom.tile([128, C], F32, tag="Mp4", name=f"Mp4_{i}")
                    for t in range(16):
                        m, j = divmod(t, 4)
                        nc.tensor.matmul(
                            Mp4[:],
                            Vm4[:, 128 * t:128 * (t + 1)],
                            U[:, 128 * m + 32 * j:128 * m + 32 * j + 32],
                            start=(t == 0),
                            stop=(t == 15),
                        )
                # skip^T via block-diagonal weights in the PE idle slot here
                # (bias rides the gelu / is folded into p1b for layer 2)
                for h in range(2):
                    nc.tensor.matmul(
                        skp[:, H * h:H * (h + 1)], pwW[i],
                        fq[:, H * h:H * (h + 1)], start=True, stop=True,
                    )
                    nc.scalar.copy(
                        skps[:, H * h:H * (h + 1)], skp[:, H * h:H * (h + 1)]
                    )
                if i == 0:
                    # build the replicated basis for the later layers in the
                    # DVE idle slot between U and the Mt4 multiplies
                    nc.vector.tensor_copy(
                        Vm4[:].rearrange("p (t r k) -> p t r k", t=16, r=4),
                        Vm[:].rearrange("p (t k) -> p t k", t=16)
                        .unsqueeze(2).broadcast_to((128, 16, 4, K)),
                    )
                    # eval stationary diag: Mt4[32j+k, 32j+c] = B[k,c] M[k,c]
                    for jj in range(4):
                        nc.vector.tensor_mul(
                            Mt4[32 * jj:32 * jj + K, 32 * jj:32 * (jj + 1)],
                            Mp[:],
                            B0,
                        )
                else:
                    nc.vector.tensor_mul(
                        Mt4[:].rearrange("p (r c) -> p r c", r=4),
                        Mp4[:].unsqueeze(1).broadcast_to((128, 4, C)),
                        B4M[i][:].rearrange("p (r c) -> p r c", r=4),
                    )
                # eval + combine, pipelined in column halves:
                # fq_next = gelu(skip + pw_b + Es * (Mt4^T @ VTs))
                PT = pev.tile([128, 512], F32, tag="PT")
                z = wp.tile([128, 512], BF16, tag="z")
                pre = wp.tile([128, 512], BF16, tag="fq")
                if i < DEPTH - 1:
                    fq2 = wp.tile([128, 512], BF16, tag="fq", name=f"fq2_{i}")
                else:
                    fq2 = pre
                for h in range(2):
                    sl = slice(H * h, H * (h + 1))
                    nc.tensor.matmul(
                        PT[:, sl], Mt4[:], VTs[:, sl], start=True, stop=True
                    )
                for h in range(2):
                    sl = slice(H * h, H * (h + 1))
                    nc.vector.tensor_mul(z[:, sl], PT[:, sl], EsT[i][:, sl])
                    nc.vector.tensor_add(pre[:, sl], z[:, sl], skps[:, sl])
                    if i < DEPTH - 1:
                        nc.scalar.activation(
                            fq2[:, sl], pre[:, sl], AF.Gelu_apprx_tanh,
                            bias=pwb[i],
                        )
                fq = fq2  # layer 2: pw_b[2] is folded into p1b on the host

            # ---------------- projection head (half-pipelined) ----------------
            pp1 = pmix.tile([128, 512], F32, tag="mix")
            g1 = wp.tile([128, 512], BF16, tag="fq")
            pp2 = pmix.tile([128, 512], F32, tag="mix")
            g2 = wp.tile([128, 512], BF16, tag="fq")
            pout = pop.tile([4, 512], F32, tag="pout")
            outsb = wp.tile([4, 512], F32, tag="outsb")
            for h in range(2):
                sl = slice(H * h, H * (h + 1))
                nc.tensor.matmul(pp1[:, sl], p1W, fq[:, sl], start=True, stop=True)
                nc.scalar.activation(g1[:, sl], pp1[:, sl], AF.Gelu_apprx_tanh, bias=p1b)
                nc.tensor.matmul(pp2[:, sl], p2W, g1[:, sl], start=True, stop=True)
                nc.scalar.activation(g2[:, sl], pp2[:, sl], AF.Gelu_apprx_tanh, bias=p2b)
                # final dot: proj3_W folded into a selection stationary
                nc.tensor.matmul(pout[:, sl], sel3, g2[:, sl], start=True, stop=True)
                nc.vector.tensor_scalar_add(outsb[:, sl], pout[:, sl], b3c)
                nc.gpsimd.dma_start(out_dram[:, sl], outsb[:, sl])

    return nc


def get_nc():
    if "nc" not in _CACHE:
        nc = bacc.Bacc("TRN2", target_bir_lowering=False, debug=False, num_devices=NCORES)
        build_program(nc)
        nc.compile()
        _CACHE["nc"] = nc
    return _CACHE["nc"]


def make_in_map(
    f_x, x_grid, q_weights, lift_W, lift_b, pw_W, pw_b, ker_log_ell, ker_log_sigma,
    proj1_W, proj1_b, proj2_W, proj2_b, proj3_W, proj3_b,
):
    f8 = lambda a: np.asarray(a, dtype=np.float64)
    x = f8(x_grid).reshape(N)
    w = f8(q_weights).reshape(N)
    f = f8(f_x).reshape(N)
    a = 0.5 * np.exp(-2.0 * f8(ker_log_ell))          # [DEPTH, C]
    sig2 = np.exp(2.0 * f8(ker_log_sigma))            # [DEPTH, C]
    ks = np.arange(K, dtype=np.float64)
    lnfact = np.concatenate([[0.0], np.cumsum(np.log(np.arange(1, K)))])

    p = np.arange(128)
    npr = np.arange(512)

    did = np.eye(128)
    d0 = np.zeros((12, D0_W), np.float64)
    for j in range(4):
        d0[3 * j:3 * j + 2, 32 * j:32 * (j + 1)] = f8(lift_W).T
        d0[3 * j + 2, 32 * j:32 * (j + 1)] = f8(lift_b)
        nn = 512 * j + npr
        d0[3 * j, 128:640] = f[nn]
        d0[3 * j + 1, 128:640] = x[nn]
        d0[3 * j + 2, 128:640] = 1.0

    def bd(W):  # block-diag lhsT: [32j+c', 32j+c] = W[c, c']
        M = np.zeros((128, 128), np.float64)
        for j in range(4):
            M[32 * j:32 * (j + 1), 32 * j:32 * (j + 1)] = f8(W).T
        return M

    def tmpN_of(i):  # w is folded into Vm', not here
        t = np.zeros((128, 512), np.float64)
        for m in range(4):
            for j in range(4):
                q = 512 * j + 128 * m + p
                t[:, 128 * m + 32 * j:128 * m + 32 * j + 32] = (
                    np.exp(-a[i][None, :] * (x[q, None] ** 2)))
        return t

    def B_of(i):  # B[k,c] = (2 a_c)^k / k!
        return np.exp(ks[:, None] * np.log(2.0 * a[i][None, :]) - lnfact[:, None])

    def B4M_of(i):
        M = np.zeros((128, 128), np.float64)
        B = B_of(i)
        for j in range(4):
            M[32 * j:32 * (j + 1), 32 * j:32 * (j + 1)] = B
        return M

    dt0 = tmpN_of(0)
    dx2 = np.zeros((128, DX2_W), np.float64)
    for j in range(4):
        dx2[32 * j:32 * (j + 1), :] = (x[None, 512 * j:512 * (j + 1)] ** 2)

    dhd = np.zeros((128, DHD_W), np.float64)
    dhd[:, 0:128] = bd(proj1_W)
    dhd[:, 128:256] = bd(proj2_W)
    for j in range(4):
        dhd[32 * j:32 * (j + 1), 256 + j] = f8(proj3_W)[0]

    dy1 = np.zeros((128, DY1_W), np.float64)
    for j in range(4):
        dy1[K * j:K * (j + 1), 0:512] = (
            x[None, 512 * j:512 * (j + 1)] ** ks[:, None])
    dy1[0:K, 512:544] = B_of(0)
    dy1[:, 544:672] = bd(pw_W[0])

    def dl_of(i):
        dl = np.zeros((128, DL_W), np.float64)
        dl[:, 0:512] = tmpN_of(i)
        dl[:, 512:640] = B4M_of(i)
        dl[:, 640:768] = bd(pw_W[i])
        return dl

    dvm = np.zeros((128, DVM_W), np.float64)
    for m in range(4):
        for j in range(4):
            q = 512 * j + 128 * m + p
            dvm[:, K * (4 * m + j):K * (4 * m + j + 1)] = (
                w[q, None] * x[q, None] ** ks[None, :])

    cfa = np.zeros((128, CF_W), np.float64)
    cfa[:, 0] = np.tile(f8(pw_b)[0], 4)
    cfa[:, 1] = np.tile(f8(pw_b)[1], 4)
    cfa[:, 2] = np.tile(f8(proj1_b) + f8(proj1_W) @ f8(pw_b)[2], 4)
    cfa[:, 3] = np.tile(f8(proj2_b), 4)
    cfa[0:4, 4] = f8(proj3_b)[0]
    for i in range(DEPTH):
        cfa[:, 5 + i] = np.tile(-a[i], 4)
        cfa[:, 8 + i] = np.tile(np.log(sig2[i]), 4)

    return {
        "did": did.astype(NPBF16),
        "d0": d0.astype(NPBF16),
        "dt0": dt0.astype(NPBF16),
        "dx2": dx2.astype(NPBF16),
        "dhd": dhd.astype(NPBF16),
        "dy1": dy1.astype(NPBF16),
        "dl1": dl_of(1).astype(NPBF16),
        "dl2": dl_of(2).astype(NPBF16),
        "dvm": dvm.astype(NPBF16),
        "cf": cfa.astype(np.float32),
    }


def kernel(**inputs) -> np.ndarray:
    nc = get_nc()
    in_map = make_in_map(**inputs)
    res = run_bass_kernel_spmd(nc, [in_map] * NCORES, list(range(NCORES)))
    return np.asarray(res.results[0]["out"], dtype=np.float32).reshape(N)


# revision 15
# speedup vs baseline: 1.1330x; 1.1330x over previous
"""Trainium2 Bass kernel for a 1D Kernel Neural Operator (KNO) on a regular grid.

Reference computation (N=2048 nodes, C=32 channels, DEPTH=3):
    fq = gelu([f_x, x] @ lift_W.T + lift_b)
    for i in 0..2:
        skip  = fq @ pw_W[i].T + pw_b[i]
        K_c   = sig2_c * exp(-(x_n - x_q)^2 * a_c),  a_c = 1/(2*ell2_c)
        integ = einsum('cnq,qc->nc', K, fq * w)
        fq    = skip + integ; gelu if i < 2
    out = (gelu(gelu(fq@W1.T+b1)@W2.T+b2)) @ W3.T + b3

Instead of materializing the C x N x N kernels, we use the factorization
exp(-a(x_n-x_q)^2) = e^{-a x_n^2} e^{2 a x_n x_q} e^{-a x_q^2} with the Taylor
expansion e^{2a x_n x_q} = sum_k (2a)^k/k! x_n^k x_q^k (K=32 terms; truncation
error < 1e-4 relative for the a-range here). Each layer's integral is then two
matmuls through the moment basis V[n,k] = x_n^k:
    U       = fq ⊙ (w_q e^{-a_c x_q^2})             [N,C]
    M[k,c]  = sum_q V[q,k] U[q,c]                   [K,C]   (PE, 16 psum-accum)
    Mt      = M ⊙ B,  B[k,c] = (2a_c)^k/k!
    integ   = (s2_c e^{-a_c x_n^2}) ⊙ (VT.T @ Mt)   [N,C]   (PE)

All basis/constant tensors (V, VT, w e^{-a x^2}, s2 e^{-a x^2}, B, block-diag
mixing weights) are pure functions of the non-f_x inputs and are precomputed on
the host; the device program is only the f_x-dependent dependency chain. All
matmul operands are bf16 (1 PE cycle/row vs 4 for fp32; rel-l2 vs the fp32
reference is ~2e-3, tolerance 2e-2); PSUM accumulation stays fp32.

Data layout: the [N,C] state lives channel-transposed in a 4-chunk stack
fqT[32j + c, n'] = fq[512j + n', c], a single [128, 512] SBUF tile. Channel
mixing (pw/proj) is ONE matmul per layer with host-built block-diagonal
[128,128] weights; biases ride the gelu activation's per-partition bias port.
The moment contraction needs q on partitions, so each layer does 4 PE
transposes of fqT back to natural layout. For layers 1-2 the moment matmuls
use a 4x-replicated basis (built once on-device by a stride-0 broadcast DVE
copy) so M lands replicated across partition groups and a single broadcast
multiply against a block-diag-masked B4 builds the [128,128] eval stationary;
layer 0 (whose replicated basis could not arrive in time by DMA) uses the
compact basis and 4 partition-shifted multiplies instead.

Scheduling: constants stream via 9 DMAs spread over the three engine DMA
rings (which run concurrently at ~140GB/s each), FIFO-ordered by need; the
gelu table prefetches during the DMA wait; eval/combine/gelu are split into
256-column halves so PE/DVE/ACT pipeline across halves; the skip matmul runs
in the PE idle slot during the Mt4 build; the skip stages to SBUF bf16 via
the idle ACT engine so the pre-add runs in the DVE 2x 16-bit mode.

Sharding: the whole problem is a dependent chain of small ops, so all 8 cores
run identical replicas (collectives cost more than they save); the output is
taken from core 0.
"""

import numpy as np
import ml_dtypes

import concourse.bass as bass
import concourse.tile as tile
from concourse import bacc, mybir
from concourse.bass_utils import run_bass_kernel_spmd

N = 2048
C = 32
K = 32
DEPTH = 3
NCORES = 8
F32 = mybir.dt.float32
BF16 = mybir.dt.bfloat16
AF = mybir.ActivationFunctionType
ALU = mybir.AluOpType
NPBF16 = ml_dtypes.bfloat16

DID_W = 128    # transpose identity
D0_W = 640     # liftT + liftin (12 partitions)
DT0_W = 512    # tmpN0
DE0_W = 512    # EsT0
DHD_W = 260    # p1W + p2W + sel3
DY1_W = 672    # VTs + B0 + pwW0
DL_W = 1280    # per-layer: tmpN + EsT + B4M + pwW  (layers 1, 2)
DVM_W = 512    # Vm
CF_W = 5
H = 256  # column half for the pipelined stages

_CACHE = {}


def build_program(nc):
    did = nc.dram_tensor("did", [128, DID_W], BF16, kind="ExternalInput")
    d0 = nc.dram_tensor("d0", [12, D0_W], BF16, kind="ExternalInput")
    dt0 = nc.dram_tensor("dt0", [128, DT0_W], BF16, kind="ExternalInput")
    de0 = nc.dram_tensor("de0", [128, DE0_W], BF16, kind="ExternalInput")
    dhd = nc.dram_tensor("dhd", [128, DHD_W], BF16, kind="ExternalInput")
    dy1 = nc.dram_tensor("dy1", [128, DY1_W], BF16, kind="ExternalInput")
    dl1 = nc.dram_tensor("dl1", [128, DL_W], BF16, kind="ExternalInput")
    dl2 = nc.dram_tensor("dl2", [128, DL_W], BF16, kind="ExternalInput")
    dvm = nc.dram_tensor("dvm", [128, DVM_W], BF16, kind="ExternalInput")
    cf = nc.dram_tensor("cf", [128, CF_W], F32, kind="ExternalInput")
    out_dram = nc.dram_tensor("out", [4, 512], F32, kind="ExternalOutput")

    with tile.TileContext(nc) as tc:
        with (
            tc.tile_pool(name="const", bufs=1) as cp,
            tc.tile_pool(name="work", bufs=3) as wp,
            tc.tile_pool(name="pmix", bufs=2, space="PSUM") as pmix,
            tc.tile_pool(name="ptr", bufs=1, space="PSUM") as ptr,
            tc.tile_pool(name="pmom", bufs=1, space="PSUM") as pmom,
            tc.tile_pool(name="pev", bufs=1, space="PSUM") as pev,
            tc.tile_pool(name="pout", bufs=1, space="PSUM") as pop,
        ):
            # warm-act source and home of the eval stationary
            Mt4 = cp.tile([128, 128], BF16, tag="Mt4")
            nc.vector.memset(Mt4[:], 0.0)

            # Constant streams: three concurrent DMA rings (one per issuing
            # engine), FIFO within a ring, ordered by first use.
            didt = cp.tile([128, DID_W], BF16, tag="didt")
            nc.scalar.dma_start(didt[:], did[:])
            d0t = cp.tile([12, D0_W], BF16, tag="d0t")
            nc.scalar.dma_start(d0t[:], d0[:])
            dt0t = cp.tile([128, DT0_W], BF16, tag="dt0t")
            nc.scalar.dma_start(dt0t[:], dt0[:])
            de0t = cp.tile([128, DE0_W], BF16, tag="de0t")
            nc.scalar.dma_start(de0t[:], de0[:])

            dy1t = cp.tile([128, DY1_W], BF16, tag="dy1t")
            nc.sync.dma_start(dy1t[:], dy1[:])
            dl1t = cp.tile([128, DL_W], BF16, tag="dl1t")
            nc.sync.dma_start(dl1t[:], dl1[:])
            dl2t = cp.tile([128, DL_W], BF16, tag="dl2t")
            nc.sync.dma_start(dl2t[:], dl2[:])
            dhdt = cp.tile([128, DHD_W], BF16, tag="dhdt")
            nc.sync.dma_start(dhdt[:], dhd[:])

            dvmt = cp.tile([128, DVM_W], BF16, tag="dvmt")
            nc.gpsimd.dma_start(dvmt[:], dvm[:])
            cft = cp.tile([128, CF_W], F32, tag="cft")
            nc.gpsimd.dma_start(cft[:], cf[:])

            # prefetch the gelu activation table during the const DMAs
            warm = wp.tile([1, 8], BF16, tag="warm")
            nc.scalar.activation(warm[:], Mt4[0:1, 0:8], AF.Gelu_apprx_tanh)

            ident = didt[:, 0:128]
            liftT = d0t[0:12, 0:128]
            liftin = d0t[0:12, 128:640]
            tmpN = [dt0t[:, 0:512], dl1t[:, 0:512], dl2t[:, 0:512]]
            EsT = [de0t[:, 0:512], dl1t[:, 512:1024], dl2t[:, 512:1024]]
            VTs = dy1t[:, 0:512]
            B0 = dy1t[0:K, 512:544]
            pwW = [dy1t[:, 544:672], dl1t[:, 1152:1280], dl2t[:, 1152:1280]]
            B4M = [None, dl1t[:, 1024:1152], dl2t[:, 1024:1152]]
            Vm = dvmt[:, 0:512]
            p1W = dhdt[:, 0:128]
            p2W = dhdt[:, 128:256]
            sel3 = dhdt[:, 256:260]
            pwb = [cft[:, 0:1], cft[:, 1:2]]
            p1b = cft[:, 2:3]
            p2b = cft[:, 3:4]
            b3c = cft[0:4, 4:5]

            # ---------------- lift ----------------
            liftp = pmix.tile([128, 512], F32, tag="mix")
            nc.tensor.matmul(liftp[:], liftT, liftin, start=True, stop=True)
            fq = wp.tile([128, 512], BF16, tag="fq")
            for h in range(2):
                nc.scalar.activation(
                    fq[:, H * h:H * (h + 1)], liftp[:, H * h:H * (h + 1)],
                    AF.Gelu_apprx_tanh,
                )

            # 4x-replicated moment basis for layers 1-2, built on-device by a
            # stride-0 broadcast copy while layer 0 runs
            Vm4 = cp.tile([128, 2048], BF16, tag="Vm4")

            # ---------------- KNO layers ----------------
            for i in range(DEPTH):
                # natural layout: trp[p, 128m + 32j + c] = fq[512j+128m+p, c]
                trp = ptr.tile([128, 512], BF16, tag="trp")
                skp = pmix.tile([128, 512], F32, tag="mix")
                U = wp.tile([128, 512], BF16, tag="U")
                skps = wp.tile([128, 512], BF16, tag="skps")
                for m in range(4):
                    nc.tensor.transpose(
                        trp[:, 128 * m:128 * (m + 1)],
                        fq[:, 128 * m:128 * (m + 1)],
                        ident,
                    )
                # U = fq_nat * (w e^{-a x^2}), halves so moments start early
                for h in range(2):
                    nc.vector.tensor_mul(
                        U[:, H * h:H * (h + 1)],
                        trp[:, H * h:H * (h + 1)],
                        tmpN[i][:, H * h:H * (h + 1)],
                    )
                if i == 0:
                    # moments M[k,c] = sum_q V[q,k] U[q,c] (compact basis)
                    Mp = pmom.tile([K, C], F32, tag="Mp")
                    for t in range(16):
                        m, j = divmod(t, 4)
                        nc.tensor.matmul(
                            Mp[:],
                            Vm[:, K * t:K * (t + 1)],
                            U[:, 128 * m + 32 * j:128 * m + 32 * j + 32],
                            start=(t == 0),
                            stop=(t == 15),
                        )
                else:
                    # replicated basis -> M lands on all 4 partition groups
                    Mp4 = pmom.tile([128, C], F32, tag="Mp4", name=f"Mp4_{i}")
                    for t in range(16):
                        m, j = divmod(t, 4)
                        nc.tensor.matmul(
                            Mp4[:],
                            Vm4[:, 128 * t:128 * (t + 1)],
                            U[:, 128 * m + 32 * j:128 * m + 32 * j + 32],
                            start=(t == 0),
                            stop=(t == 15),
                        )
                # skip^T via block-diagonal weights in the PE idle slot here
                # (bias rides the gelu / is folded into p1b for layer 2)
                for h in range(2):
                    nc.tensor.matmul(
                        skp[:, H * h:H * (h + 1)], pwW[i],
                        fq[:, H * h:H * (h + 1)], start=True, stop=True,
                    )
                    nc.scalar.copy(
                        skps[:, H * h:H * (h + 1)], skp[:, H * h:H * (h + 1)]
                    )
                if i == 0:
                    # build the replicated basis for the later layers in the
                    # DVE idle slot between U and the Mt4 multiplies
                    nc.vector.tensor_copy(
                        Vm4[:].rearrange("p (t r k) -> p t r k", t=16, r=4),
                        Vm[:].rearrange("p (t k) -> p t k", t=16)
                        .unsqueeze(2).broadcast_to((128, 16, 4, K)),
                    )
                    # eval stationary diag: Mt4[32j+k, 32j+c] = B[k,c] M[k,c]
                    for jj in range(4):
                        nc.vector.tensor_mul(
                            Mt4[32 * jj:32 * jj + K, 32 * jj:32 * (jj + 1)],
                            Mp[:],
                            B0,
                        )
                else:
                    nc.vector.tensor_mul(
                        Mt4[:].rearrange("p (r c) -> p r c", r=4),
                        Mp4[:].unsqueeze(1).broadcast_to((128, 4, C)),
                        B4M[i][:].rearrange("p (r c) -> p r c", r=4),
                    )
                # eval + combine, pipelined in column halves:
                # fq_next = gelu(skip + pw_b + Es * (Mt4^T @ VTs))
                PT = pev.tile([128, 512], F32, tag="PT")
                z = wp.tile([128, 512], BF16, tag="z")
                pre = wp.tile([128, 512], BF16, tag="fq")
                if i < DEPTH - 1:
                    fq2 = wp.tile([128, 512], BF16, tag="fq", name=f"fq2_{i}")
                else:
                    fq2 = pre
                for h in range(2):
                    sl = slice(H * h, H * (h + 1))
                    nc.tensor.matmul(
                        PT[:, sl], Mt4[:], VTs[:, sl], start=True, stop=True
                    )
                for h in range(2):
                    sl = slice(H * h, H * (h + 1))
                    nc.vector.tensor_mul(z[:, sl], PT[:, sl], EsT[i][:, sl])
                    nc.vector.tensor_add(pre[:, sl], z[:, sl], skps[:, sl])
                    if i < DEPTH - 1:
                        nc.scalar.activation(
                            fq2[:, sl], pre[:, sl], AF.Gelu_apprx_tanh,
                            bias=pwb[i],
                        )
                fq = fq2  # layer 2: pw_b[2] is folded into p1b on the host

            # ---------------- projection head (half-pipelined) ----------------
            pp1 = pmix.tile([128, 512], F32, tag="mix")
            g1 = wp.tile([128, 512], BF16, tag="fq")
            pp2 = pmix.tile([128, 512], F32, tag="mix")
            g2 = wp.tile([128, 512], BF16, tag="fq")
            pout = pop.tile([4, 512], F32, tag="pout")
            outsb = wp.tile([4, 512], F32, tag="outsb")
            for h in range(2):
                sl = slice(H * h, H * (h + 1))
                nc.tensor.matmul(pp1[:, sl], p1W, fq[:, sl], start=True, stop=True)
                nc.scalar.activation(g1[:, sl], pp1[:, sl], AF.Gelu_apprx_tanh, bias=p1b)
                nc.tensor.matmul(pp2[:, sl], p2W, g1[:, sl], start=True, stop=True)
                nc.scalar.activation(g2[:, sl], pp2[:, sl], AF.Gelu_apprx_tanh, bias=p2b)
                # final dot: proj3_W folded into a selection stationary
                nc.tensor.matmul(pout[:, sl], sel3, g2[:, sl], start=True, stop=True)
                nc.vector.tensor_scalar_add(outsb[:, sl], pout[:, sl], b3c)
                nc.gpsimd.dma_start(out_dram[:, sl], outsb[:, sl])

    return nc


def get_nc():
    if "nc" not in _CACHE:
        nc = bacc.Bacc("TRN2", target_bir_lowering=False, debug=False, num_devices=NCORES)
        build_program(nc)
        nc.compile()
        _CACHE["nc"] = nc
    return _CACHE["nc"]


def make_in_map(
    f_x, x_grid, q_weights, lift_W, lift_b, pw_W, pw_b, ker_log_ell, ker_log_sigma,
    proj1_W, proj1_b, proj2_W, proj2_b, proj3_W, proj3_b,
):
    f8 = lambda a: np.asarray(a, dtype=np.float64)
    x = f8(x_grid).reshape(N)
    w = f8(q_weights).reshape(N)
    f = f8(f_x).reshape(N)
    a = 0.5 * np.exp(-2.0 * f8(ker_log_ell))          # [DEPTH, C]
    sig2 = np.exp(2.0 * f8(ker_log_sigma))            # [DEPTH, C]
    ks = np.arange(K, dtype=np.float64)
    lnfact = np.concatenate([[0.0], np.cumsum(np.log(np.arange(1, K)))])

    p = np.arange(128)
    npr = np.arange(512)

    did = np.eye(128)
    d0 = np.zeros((12, D0_W), np.float64)
    for j in range(4):
        d0[3 * j:3 * j + 2, 32 * j:32 * (j + 1)] = f8(lift_W).T
        d0[3 * j + 2, 32 * j:32 * (j + 1)] = f8(lift_b)
        nn = 512 * j + npr
        d0[3 * j, 128:640] = f[nn]
        d0[3 * j + 1, 128:640] = x[nn]
        d0[3 * j + 2, 128:640] = 1.0

    def bd(W):  # block-diag lhsT: [32j+c', 32j+c] = W[c, c']
        M = np.zeros((128, 128), np.float64)
        for j in range(4):
            M[32 * j:32 * (j + 1), 32 * j:32 * (j + 1)] = f8(W).T
        return M

    def tmpN_of(i):
        t = np.zeros((128, 512), np.float64)
        for m in range(4):
            for j in range(4):
                q = 512 * j + 128 * m + p
                t[:, 128 * m + 32 * j:128 * m + 32 * j + 32] = (
                    w[q, None] * np.exp(-a[i][None, :] * (x[q, None] ** 2)))
        return t

    def EsT_of(i):
        e = np.zeros((128, 512), np.float64)
        for j in range(4):
            nn = 512 * j + npr
            e[32 * j:32 * (j + 1), :] = (
                sig2[i][:, None] * np.exp(-a[i][:, None] * (x[None, nn] ** 2)))
        return e

    def B_of(i):  # B[k,c] = (2 a_c)^k / k!
        return np.exp(ks[:, None] * np.log(2.0 * a[i][None, :]) - lnfact[:, None])

    def B4M_of(i):
        M = np.zeros((128, 128), np.float64)
        B = B_of(i)
        for j in range(4):
            M[32 * j:32 * (j + 1), 32 * j:32 * (j + 1)] = B
        return M

    dt0 = tmpN_of(0)
    de0 = EsT_of(0)

    dhd = np.zeros((128, DHD_W), np.float64)
    dhd[:, 0:128] = bd(proj1_W)
    dhd[:, 128:256] = bd(proj2_W)
    for j in range(4):
        dhd[32 * j:32 * (j + 1), 256 + j] = f8(proj3_W)[0]

    dy1 = np.zeros((128, DY1_W), np.float64)
    for j in range(4):
        dy1[K * j:K * (j + 1), 0:512] = (
            x[None, 512 * j:512 * (j + 1)] ** ks[:, None])
    dy1[0:K, 512:544] = B_of(0)
    dy1[:, 544:672] = bd(pw_W[0])

    def dl_of(i):
        dl = np.zeros((128, DL_W), np.float64)
        dl[:, 0:512] = tmpN_of(i)
        dl[:, 512:1024] = EsT_of(i)
        dl[:, 1024:1152] = B4M_of(i)
        dl[:, 1152:1280] = bd(pw_W[i])
        return dl

    dvm = np.zeros((128, DVM_W), np.float64)
    for m in range(4):
        for j in range(4):
            q = 512 * j + 128 * m + p
            dvm[:, K * (4 * m + j):K * (4 * m + j + 1)] = x[q, None] ** ks[None, :]

    cfa = np.zeros((128, CF_W), np.float64)
    cfa[:, 0] = np.tile(f8(pw_b)[0], 4)
    cfa[:, 1] = np.tile(f8(pw_b)[1], 4)
    cfa[:, 2] = np.tile(f8(proj1_b) + f8(proj1_W) @ f8(pw_b)[2], 4)
    cfa[:, 3] = np.tile(f8(proj2_b), 4)
    cfa[0:4, 4] = f8(proj3_b)[0]

    return {
        "did": did.astype(NPBF16),
        "d0": d0.astype(NPBF16),
        "dt0": dt0.astype(NPBF16),
        "de0": de0.astype(NPBF16),
        "dhd": dhd.astype(NPBF16),
        "dy1": dy1.astype(NPBF16),
        "dl1": dl_of(1).astype(NPBF16),
        "dl2": dl_of(2).astype(NPBF16),
        "dvm": dvm.astype(NPBF16),
        "cf": cfa.astype(np.float32),
    }


def kernel(**inputs) -> np.ndarray:
    nc = get_nc()
    in_map = make_in_map(**inputs)
    res = run_bass_kernel_spmd(nc, [in_map] * NCORES, list(range(NCORES)))
    return np.asarray(res.results[0]["out"], dtype=np.float32).reshape(N)


# revision 16
# speedup vs baseline: 1.1487x; 1.0139x over previous
"""Trainium2 Bass kernel for a 1D Kernel Neural Operator (KNO) on a regular grid.

Reference computation (N=2048 nodes, C=32 channels, DEPTH=3):
    fq = gelu([f_x, x] @ lift_W.T + lift_b)
    for i in 0..2:
        skip  = fq @ pw_W[i].T + pw_b[i]
        K_c   = sig2_c * exp(-(x_n - x_q)^2 * a_c),  a_c = 1/(2*ell2_c)
        integ = einsum('cnq,qc->nc', K, fq * w)
        fq    = skip + integ; gelu if i < 2
    out = (gelu(gelu(fq@W1.T+b1)@W2.T+b2)) @ W3.T + b3

Instead of materializing the C x N x N kernels, we use the factorization
exp(-a(x_n-x_q)^2) = e^{-a x_n^2} e^{2 a x_n x_q} e^{-a x_q^2} with the Taylor
expansion e^{2a x_n x_q} = sum_k (2a)^k/k! x_n^k x_q^k (K=32 terms; truncation
error < 1e-4 relative for the a-range here). Each layer's integral is then two
matmuls through the moment basis V[n,k] = x_n^k:
    U       = fq ⊙ (w_q e^{-a_c x_q^2})             [N,C]
    M[k,c]  = sum_q V[q,k] U[q,c]                   [K,C]   (PE, 16 psum-accum)
    Mt      = M ⊙ B,  B[k,c] = (2a_c)^k/k!
    integ   = (s2_c e^{-a_c x_n^2}) ⊙ (VT.T @ Mt)   [N,C]   (PE)

All basis/constant tensors (V, VT, w e^{-a x^2}, s2 e^{-a x^2}, B, block-diag
mixing weights) are pure functions of the non-f_x inputs and are precomputed on
the host; the device program is only the f_x-dependent dependency chain. All
matmul operands are bf16 (1 PE cycle/row vs 4 for fp32; rel-l2 vs the fp32
reference is ~2e-3, tolerance 2e-2); PSUM accumulation stays fp32.

Data layout: the [N,C] state lives channel-transposed in a 4-chunk stack
fqT[32j + c, n'] = fq[512j + n', c], a single [128, 512] SBUF tile. Channel
mixing (pw/proj) is ONE matmul per layer with host-built block-diagonal
[128,128] weights; biases ride the gelu activation's per-partition bias port.
The moment contraction needs q on partitions, so each layer does 4 PE
transposes of fqT back to natural layout. For layers 1-2 the moment matmuls
use a 4x-replicated basis (built once on-device by a stride-0 broadcast DVE
copy) so M lands replicated across partition groups and a single broadcast
multiply against a block-diag-masked B4 builds the [128,128] eval stationary;
layer 0 (whose replicated basis could not arrive in time by DMA) uses the
compact basis and 4 partition-shifted multiplies instead.

Scheduling: constants stream via 9 DMAs spread over the three engine DMA
rings (which run concurrently at ~140GB/s each), FIFO-ordered by need; the
gelu table prefetches during the DMA wait; eval/combine/gelu are split into
256-column halves so PE/DVE/ACT pipeline across halves; the skip matmul runs
in the PE idle slot during the Mt4 build; the skip stages to SBUF bf16 via
the idle ACT engine so the pre-add runs in the DVE 2x 16-bit mode.

Sharding: the whole problem is a dependent chain of small ops, so all 8 cores
run identical replicas (collectives cost more than they save); the output is
taken from core 0.
"""

import numpy as np
import ml_dtypes

import concourse.bass as bass
import concourse.tile as tile
from concourse import bacc, mybir
from concourse.bass_utils import run_bass_kernel_spmd

N = 2048
C = 32
K = 32
DEPTH = 3
NCORES = 8
F32 = mybir.dt.float32
BF16 = mybir.dt.bfloat16
AF = mybir.ActivationFunctionType
ALU = mybir.AluOpType
NPBF16 = ml_dtypes.bfloat16

DID_W = 128    # transpose identity
D0_W = 640     # liftT + liftin (12 partitions)
DT0_W = 512    # tmpN0
DE0_W = 512    # EsT0
DHD_W = 260    # p1W + p2W + sel3
DY1_W = 768    # VTs + B4M0 + pwW0
DL_W = 1280    # per-layer: tmpN + EsT + B4M + pwW  (layers 1, 2)
DVM_W = 512    # Vm
CF_W = 5
H = 256  # column half for the pipelined stages

_CACHE = {}


def build_program(nc):
    did = nc.dram_tensor("did", [128, DID_W], BF16, kind="ExternalInput")
    d0 = nc.dram_tensor("d0", [12, D0_W], BF16, kind="ExternalInput")
    dt0 = nc.dram_tensor("dt0", [128, DT0_W], BF16, kind="ExternalInput")
    de0 = nc.dram_tensor("de0", [128, DE0_W], BF16, kind="ExternalInput")
    dhd = nc.dram_tensor("dhd", [128, DHD_W], BF16, kind="ExternalInput")
    dy1 = nc.dram_tensor("dy1", [128, DY1_W], BF16, kind="ExternalInput")
    dl1 = nc.dram_tensor("dl1", [128, DL_W], BF16, kind="ExternalInput")
    dl2 = nc.dram_tensor("dl2", [128, DL_W], BF16, kind="ExternalInput")
    dvm = nc.dram_tensor("dvm", [128, DVM_W], BF16, kind="ExternalInput")
    cf = nc.dram_tensor("cf", [128, CF_W], F32, kind="ExternalInput")
    out_dram = nc.dram_tensor("out", [4, 512], F32, kind="ExternalOutput")

    with tile.TileContext(nc) as tc:
        with (
            tc.tile_pool(name="const", bufs=1) as cp,
            tc.tile_pool(name="work", bufs=3) as wp,
            tc.tile_pool(name="pmix", bufs=2, space="PSUM") as pmix,
            tc.tile_pool(name="ptr", bufs=1, space="PSUM") as ptr,
            tc.tile_pool(name="pmom", bufs=1, space="PSUM") as pmom,
            tc.tile_pool(name="pev", bufs=1, space="PSUM") as pev,
            tc.tile_pool(name="pout", bufs=1, space="PSUM") as pop,
        ):
            # warm-act source and home of the eval stationary
            Mt4 = cp.tile([128, 128], BF16, tag="Mt4")
            nc.vector.memset(Mt4[:], 0.0)

            # Constant streams: three concurrent DMA rings (one per issuing
            # engine), FIFO within a ring, ordered by first use.
            d0t = cp.tile([12, D0_W], BF16, tag="d0t")
            nc.scalar.dma_start(d0t[:], d0[:])
            didt = cp.tile([128, DID_W], BF16, tag="didt")
            nc.scalar.dma_start(didt[:], did[:])
            dt0t = cp.tile([128, DT0_W], BF16, tag="dt0t")
            nc.scalar.dma_start(dt0t[:], dt0[:])
            de0t = cp.tile([128, DE0_W], BF16, tag="de0t")
            nc.scalar.dma_start(de0t[:], de0[:])

            dy1t = cp.tile([128, DY1_W], BF16, tag="dy1t")
            nc.sync.dma_start(dy1t[:], dy1[:])
            dl1t = cp.tile([128, DL_W], BF16, tag="dl1t")
            nc.sync.dma_start(dl1t[:], dl1[:])
            dl2t = cp.tile([128, DL_W], BF16, tag="dl2t")
            nc.sync.dma_start(dl2t[:], dl2[:])
            dhdt = cp.tile([128, DHD_W], BF16, tag="dhdt")
            nc.sync.dma_start(dhdt[:], dhd[:])

            dvmt = cp.tile([128, DVM_W], BF16, tag="dvmt")
            nc.gpsimd.dma_start(dvmt[:], dvm[:])
            cft = cp.tile([128, CF_W], F32, tag="cft")
            nc.gpsimd.dma_start(cft[:], cf[:])

            # prefetch the gelu activation table during the const DMAs
            warm = wp.tile([1, 8], BF16, tag="warm")
            nc.scalar.activation(warm[:], Mt4[0:1, 0:8], AF.Gelu_apprx_tanh)

            ident = didt[:, 0:128]
            liftT = d0t[0:12, 0:128]
            liftin = d0t[0:12, 128:640]
            tmpN = [dt0t[:, 0:512], dl1t[:, 0:512], dl2t[:, 0:512]]
            EsT = [de0t[:, 0:512], dl1t[:, 512:1024], dl2t[:, 512:1024]]
            VTs = dy1t[:, 0:512]
            pwW = [dy1t[:, 640:768], dl1t[:, 1152:1280], dl2t[:, 1152:1280]]
            B4M = [dy1t[:, 512:640], dl1t[:, 1024:1152], dl2t[:, 1024:1152]]
            Vm = dvmt[:, 0:512]
            p1W = dhdt[:, 0:128]
            p2W = dhdt[:, 128:256]
            sel3 = dhdt[:, 256:260]
            pwb = [cft[:, 0:1], cft[:, 1:2]]
            p1b = cft[:, 2:3]
            p2b = cft[:, 3:4]
            b3c = cft[0:4, 4:5]

            # ---------------- lift ----------------
            liftp = pmix.tile([128, 512], F32, tag="mix")
            nc.tensor.matmul(liftp[:], liftT, liftin, start=True, stop=True)
            fq = wp.tile([128, 512], BF16, tag="fq")
            for h in range(2):
                nc.scalar.activation(
                    fq[:, H * h:H * (h + 1)], liftp[:, H * h:H * (h + 1)],
                    AF.Gelu_apprx_tanh,
                )

            # 4x-replicated moment basis, built on-device by a stride-0
            # broadcast copy as soon as Vm lands (DVE is idle then)
            Vm4 = cp.tile([128, 2048], BF16, tag="Vm4")
            nc.vector.tensor_copy(
                Vm4[:].rearrange("p (t r k) -> p t r k", t=16, r=4),
                Vm[:].rearrange("p (t k) -> p t k", t=16)
                .unsqueeze(2).broadcast_to((128, 16, 4, K)),
            )

            # ---------------- KNO layers ----------------
            for i in range(DEPTH):
                # natural layout: trp[p, 128m + 32j + c] = fq[512j+128m+p, c]
                trp = ptr.tile([128, 512], BF16, tag="trp")
                skp = pmix.tile([128, 512], F32, tag="mix")
                U = wp.tile([128, 512], BF16, tag="U")
                skps = wp.tile([128, 512], BF16, tag="skps")
                for m in range(4):
                    nc.tensor.transpose(
                        trp[:, 128 * m:128 * (m + 1)],
                        fq[:, 128 * m:128 * (m + 1)],
                        ident,
                    )
                # U = fq_nat * (w e^{-a x^2}), halves so moments start early
                for h in range(2):
                    nc.vector.tensor_mul(
                        U[:, H * h:H * (h + 1)],
                        trp[:, H * h:H * (h + 1)],
                        tmpN[i][:, H * h:H * (h + 1)],
                    )
                # replicated basis -> M lands on all 4 partition groups
                Mp4 = pmom.tile([128, C], F32, tag="Mp4", name=f"Mp4_{i}")
                for t in range(16):
                    m, j = divmod(t, 4)
                    nc.tensor.matmul(
                        Mp4[:],
                        Vm4[:, 128 * t:128 * (t + 1)],
                        U[:, 128 * m + 32 * j:128 * m + 32 * j + 32],
                        start=(t == 0),
                        stop=(t == 15),
                    )
                # skip^T via block-diagonal weights in the PE idle slot here
                # (bias rides the gelu / is folded into p1b for layer 2)
                for h in range(2):
                    nc.tensor.matmul(
                        skp[:, H * h:H * (h + 1)], pwW[i],
                        fq[:, H * h:H * (h + 1)], start=True, stop=True,
                    )
                    nc.scalar.copy(
                        skps[:, H * h:H * (h + 1)], skp[:, H * h:H * (h + 1)]
                    )
                nc.vector.tensor_mul(
                    Mt4[:].rearrange("p (r c) -> p r c", r=4),
                    Mp4[:].unsqueeze(1).broadcast_to((128, 4, C)),
                    B4M[i][:].rearrange("p (r c) -> p r c", r=4),
                )
                # eval + combine, pipelined in column halves:
                # fq_next = gelu(skip + pw_b + Es * (Mt4^T @ VTs))
                PT = pev.tile([128, 512], F32, tag="PT")
                z = wp.tile([128, 512], BF16, tag="z")
                pre = wp.tile([128, 512], BF16, tag="fq")
                if i < DEPTH - 1:
                    fq2 = wp.tile([128, 512], BF16, tag="fq", name=f"fq2_{i}")
                else:
                    fq2 = pre
                for h in range(2):
                    sl = slice(H * h, H * (h + 1))
                    nc.tensor.matmul(
                        PT[:, sl], Mt4[:], VTs[:, sl], start=True, stop=True
                    )
                for h in range(2):
                    sl = slice(H * h, H * (h + 1))
                    nc.vector.tensor_mul(z[:, sl], PT[:, sl], EsT[i][:, sl])
                    nc.vector.tensor_add(pre[:, sl], z[:, sl], skps[:, sl])
                    if i < DEPTH - 1:
                        nc.scalar.activation(
                            fq2[:, sl], pre[:, sl], AF.Gelu_apprx_tanh,
                            bias=pwb[i],
                        )
                fq = fq2  # layer 2: pw_b[2] is folded into p1b on the host

            # ---------------- projection head (half-pipelined) ----------------
            pp1 = pmix.tile([128, 512], F32, tag="mix")
            g1 = wp.tile([128, 512], BF16, tag="fq")
            pp2 = pmix.tile([128, 512], F32, tag="mix")
            g2 = wp.tile([128, 512], BF16, tag="fq")
            pout = pop.tile([4, 512], F32, tag="pout")
            outsb = wp.tile([4, 512], F32, tag="outsb")
            for h in range(2):
                sl = slice(H * h, H * (h + 1))
                nc.tensor.matmul(pp1[:, sl], p1W, fq[:, sl], start=True, stop=True)
                nc.scalar.activation(g1[:, sl], pp1[:, sl], AF.Gelu_apprx_tanh, bias=p1b)
                nc.tensor.matmul(pp2[:, sl], p2W, g1[:, sl], start=True, stop=True)
                nc.scalar.activation(g2[:, sl], pp2[:, sl], AF.Gelu_apprx_tanh, bias=p2b)
                # final dot: proj3_W folded into a selection stationary
                nc.tensor.matmul(pout[:, sl], sel3, g2[:, sl], start=True, stop=True)
                nc.vector.tensor_scalar_add(outsb[:, sl], pout[:, sl], b3c)
                eng = nc.gpsimd if h == 0 else nc.sync
                eng.dma_start(out_dram[:, sl], outsb[:, sl])

    return nc


def get_nc():
    if "nc" not in _CACHE:
        nc = bacc.Bacc("TRN2", target_bir_lowering=False, debug=False, num_devices=NCORES)
        build_program(nc)
        nc.compile()
        _CACHE["nc"] = nc
    return _CACHE["nc"]


def make_in_map(
    f_x, x_grid, q_weights, lift_W, lift_b, pw_W, pw_b, ker_log_ell, ker_log_sigma,
    proj1_W, proj1_b, proj2_W, proj2_b, proj3_W, proj3_b,
):
    f8 = lambda a: np.asarray(a, dtype=np.float64)
    x = f8(x_grid).reshape(N)
    w = f8(q_weights).reshape(N)
    f = f8(f_x).reshape(N)
    a = 0.5 * np.exp(-2.0 * f8(ker_log_ell))          # [DEPTH, C]
    sig2 = np.exp(2.0 * f8(ker_log_sigma))            # [DEPTH, C]
    ks = np.arange(K, dtype=np.float64)
    lnfact = np.concatenate([[0.0], np.cumsum(np.log(np.arange(1, K)))])

    p = np.arange(128)
    npr = np.arange(512)

    did = np.eye(128)
    d0 = np.zeros((12, D0_W), np.float64)
    for j in range(4):
        d0[3 * j:3 * j + 2, 32 * j:32 * (j + 1)] = f8(lift_W).T
        d0[3 * j + 2, 32 * j:32 * (j + 1)] = f8(lift_b)
        nn = 512 * j + npr
        d0[3 * j, 128:640] = f[nn]
        d0[3 * j + 1, 128:640] = x[nn]
        d0[3 * j + 2, 128:640] = 1.0

    def bd(W):  # block-diag lhsT: [32j+c', 32j+c] = W[c, c']
        M = np.zeros((128, 128), np.float64)
        for j in range(4):
            M[32 * j:32 * (j + 1), 32 * j:32 * (j + 1)] = f8(W).T
        return M

    def tmpN_of(i):
        t = np.zeros((128, 512), np.float64)
        for m in range(4):
            for j in range(4):
                q = 512 * j + 128 * m + p
                t[:, 128 * m + 32 * j:128 * m + 32 * j + 32] = (
                    w[q, None] * np.exp(-a[i][None, :] * (x[q, None] ** 2)))
        return t

    def EsT_of(i):
        e = np.zeros((128, 512), np.float64)
        for j in range(4):
            nn = 512 * j + npr
            e[32 * j:32 * (j + 1), :] = (
                sig2[i][:, None] * np.exp(-a[i][:, None] * (x[None, nn] ** 2)))
        return e

    def B_of(i):  # B[k,c] = (2 a_c)^k / k!
        return np.exp(ks[:, None] * np.log(2.0 * a[i][None, :]) - lnfact[:, None])

    def B4M_of(i):
        M = np.zeros((128, 128), np.float64)
        B = B_of(i)
        for j in range(4):
            M[32 * j:32 * (j + 1), 32 * j:32 * (j + 1)] = B
        return M

    dt0 = tmpN_of(0)
    de0 = EsT_of(0)

    dhd = np.zeros((128, DHD_W), np.float64)
    dhd[:, 0:128] = bd(proj1_W)
    dhd[:, 128:256] = bd(proj2_W)
    for j in range(4):
        dhd[32 * j:32 * (j + 1), 256 + j] = f8(proj3_W)[0]

    dy1 = np.zeros((128, DY1_W), np.float64)
    for j in range(4):
        dy1[K * j:K * (j + 1), 0:512] = (
            x[None, 512 * j:512 * (j + 1)] ** ks[:, None])
    dy1[:, 512:640] = B4M_of(0)
    dy1[:, 640:768] = bd(pw_W[0])

    def dl_of(i):
        dl = np.zeros((128, DL_W), np.float64)
        dl[:, 0:512] = tmpN_of(i)
        dl[:, 512:1024] = EsT_of(i)
        dl[:, 1024:1152] = B4M_of(i)
        dl[:, 1152:1280] = bd(pw_W[i])
        return dl

    dvm = np.zeros((128, DVM_W), np.float64)
    for m in range(4):
        for j in range(4):
            q = 512 * j + 128 * m + p
            dvm[:, K * (4 * m + j):K * (4 * m + j + 1)] = x[q, None] ** ks[None, :]

    cfa = np.zeros((128, CF_W), np.float64)
    cfa[:, 0] = np.tile(f8(pw_b)[0], 4)
    cfa[:, 1] = np.tile(f8(pw_b)[1], 4)
    cfa[:, 2] = np.tile(f8(proj1_b) + f8(proj1_W) @ f8(pw_b)[2], 4)
    cfa[:, 3] = np.tile(f8(proj2_b), 4)
    cfa[0:4, 4] = f8(proj3_b)[0]

    return {
        "did": did.astype(NPBF16),
        "d0": d0.astype(NPBF16),
        "dt0": dt0.astype(NPBF16),
        "de0": de0.astype(NPBF16),
        "dhd": dhd.astype(NPBF16),
        "dy1": dy1.astype(NPBF16),
        "dl1": dl_of(1).astype(NPBF16),
        "dl2": dl_of(2).astype(NPBF16),
        "dvm": dvm.astype(NPBF16),
        "cf": cfa.astype(np.float32),
    }


def kernel(**inputs) -> np.ndarray:
    nc = get_nc()
    in_map = make_in_map(**inputs)
    res = run_bass_kernel_spmd(nc, [in_map] * NCORES, list(range(NCORES)))
    return np.asarray(res.results[0]["out"], dtype=np.float32).reshape(N)


# revision 17
# speedup vs baseline: 1.1510x; 1.0020x over previous
"""Trainium2 Bass kernel for a 1D Kernel Neural Operator (KNO) on a regular grid.

Reference computation (N=2048 nodes, C=32 channels, DEPTH=3):
    fq = gelu([f_x, x] @ lift_W.T + lift_b)
    for i in 0..2:
        skip  = fq @ pw_W[i].T + pw_b[i]
        K_c   = sig2_c * exp(-(x_n - x_q)^2 * a_c),  a_c = 1/(2*ell2_c)
        integ = einsum('cnq,qc->nc', K, fq * w)
        fq    = skip + integ; gelu if i < 2
    out = (gelu(gelu(fq@W1.T+b1)@W2.T+b2)) @ W3.T + b3

Instead of materializing the C x N x N kernels, we use the factorization
exp(-a(x_n-x_q)^2) = e^{-a x_n^2} e^{2 a x_n x_q} e^{-a x_q^2} with the Taylor
expansion e^{2a x_n x_q} = sum_k (2a)^k/k! x_n^k x_q^k (K=32 terms; truncation
error < 1e-4 relative for the a-range here). Each layer's integral is then two
matmuls through the moment basis V[n,k] = x_n^k:
    U       = fq ⊙ (w_q e^{-a_c x_q^2})             [N,C]
    M[k,c]  = sum_q V[q,k] U[q,c]                   [K,C]   (PE, 16 psum-accum)
    Mt      = M ⊙ B,  B[k,c] = (2a_c)^k/k!
    integ   = (s2_c e^{-a_c x_n^2}) ⊙ (VT.T @ Mt)   [N,C]   (PE)

All basis/constant tensors (V, VT, w e^{-a x^2}, s2 e^{-a x^2}, B, block-diag
mixing weights) are pure functions of the non-f_x inputs and are precomputed on
the host; the device program is only the f_x-dependent dependency chain. All
matmul operands are bf16 (1 PE cycle/row vs 4 for fp32; rel-l2 vs the fp32
reference is ~2e-3, tolerance 2e-2); PSUM accumulation stays fp32.

Data layout: the [N,C] state lives channel-transposed in a 4-chunk stack
fqT[32j + c, n'] = fq[512j + n', c], a single [128, 512] SBUF tile. Channel
mixing (pw/proj) is ONE matmul per layer with host-built block-diagonal
[128,128] weights; biases ride the gelu activation's per-partition bias port.
The moment contraction needs q on partitions, so each layer does 4 PE
transposes of fqT back to natural layout. For layers 1-2 the moment matmuls
use a 4x-replicated basis (built once on-device by a stride-0 broadcast DVE
copy) so M lands replicated across partition groups and a single broadcast
multiply against a block-diag-masked B4 builds the [128,128] eval stationary;
layer 0 (whose replicated basis could not arrive in time by DMA) uses the
compact basis and 4 partition-shifted multiplies instead.

Scheduling: constants stream via 9 DMAs spread over the three engine DMA
rings (which run concurrently at ~140GB/s each), FIFO-ordered by need; the
gelu table prefetches during the DMA wait; eval/combine/gelu are split into
256-column halves so PE/DVE/ACT pipeline across halves; the skip matmul runs
in the PE idle slot during the Mt4 build; the skip stages to SBUF bf16 via
the idle ACT engine so the pre-add runs in the DVE 2x 16-bit mode.

Sharding: the whole problem is a dependent chain of small ops, so all 8 cores
run identical replicas (collectives cost more than they save); the output is
taken from core 0.
"""

import numpy as np
import ml_dtypes

import concourse.bass as bass
import concourse.tile as tile
from concourse import bacc, mybir
from concourse.bass_utils import run_bass_kernel_spmd

N = 2048
C = 32
K = 32
DEPTH = 3
NCORES = 8
F32 = mybir.dt.float32
BF16 = mybir.dt.bfloat16
AF = mybir.ActivationFunctionType
ALU = mybir.AluOpType
NPBF16 = ml_dtypes.bfloat16

DID_W = 128    # transpose identity
D0_W = 640     # liftT + liftin (12 partitions)
DT0_W = 512    # tmpN0
DE0_W = 512    # EsT0
DHD_W = 260    # p1W + p2W + sel3
DY1_W = 768    # VTs + B4M0 + pwW0
DL_W = 1280    # per-layer: tmpN + EsT + B4M + pwW  (layers 1, 2)
DVM_W = 512    # Vm
CF_W = 5
H = 256  # column half for the pipelined stages
HSPLIT = [(0, 384), (384, 512)]  # asymmetric halves: short trailing chain

_CACHE = {}


def build_program(nc):
    did = nc.dram_tensor("did", [128, DID_W], BF16, kind="ExternalInput")
    d0 = nc.dram_tensor("d0", [12, D0_W], BF16, kind="ExternalInput")
    dt0 = nc.dram_tensor("dt0", [128, DT0_W], BF16, kind="ExternalInput")
    de0 = nc.dram_tensor("de0", [128, DE0_W], BF16, kind="ExternalInput")
    dhd = nc.dram_tensor("dhd", [128, DHD_W], BF16, kind="ExternalInput")
    dy1 = nc.dram_tensor("dy1", [128, DY1_W], BF16, kind="ExternalInput")
    dl1 = nc.dram_tensor("dl1", [128, DL_W], BF16, kind="ExternalInput")
    dl2 = nc.dram_tensor("dl2", [128, DL_W], BF16, kind="ExternalInput")
    dvm = nc.dram_tensor("dvm", [128, DVM_W], BF16, kind="ExternalInput")
    cf = nc.dram_tensor("cf", [128, CF_W], F32, kind="ExternalInput")
    out_dram = nc.dram_tensor("out", [4, 512], F32, kind="ExternalOutput")

    with tile.TileContext(nc) as tc:
        with (
            tc.tile_pool(name="const", bufs=1) as cp,
            tc.tile_pool(name="work", bufs=3) as wp,
            tc.tile_pool(name="pmix", bufs=2, space="PSUM") as pmix,
            tc.tile_pool(name="ptr", bufs=1, space="PSUM") as ptr,
            tc.tile_pool(name="pmom", bufs=1, space="PSUM") as pmom,
            tc.tile_pool(name="pev", bufs=1, space="PSUM") as pev,
            tc.tile_pool(name="pout", bufs=1, space="PSUM") as pop,
        ):
            # warm-act source and home of the eval stationary
            Mt4 = cp.tile([128, 128], BF16, tag="Mt4")
            nc.vector.memset(Mt4[:], 0.0)

            # Constant streams: three concurrent DMA rings (one per issuing
            # engine), FIFO within a ring, ordered by first use.
            d0t = cp.tile([12, D0_W], BF16, tag="d0t")
            nc.scalar.dma_start(d0t[:], d0[:])
            didt = cp.tile([128, DID_W], BF16, tag="didt")
            nc.scalar.dma_start(didt[:], did[:])
            dt0t = cp.tile([128, DT0_W], BF16, tag="dt0t")
            nc.scalar.dma_start(dt0t[:], dt0[:])
            de0t = cp.tile([128, DE0_W], BF16, tag="de0t")
            nc.scalar.dma_start(de0t[:], de0[:])

            dy1t = cp.tile([128, DY1_W], BF16, tag="dy1t")
            nc.sync.dma_start(dy1t[:], dy1[:])
            dl1t = cp.tile([128, DL_W], BF16, tag="dl1t")
            nc.sync.dma_start(dl1t[:], dl1[:])
            dl2t = cp.tile([128, DL_W], BF16, tag="dl2t")
            nc.sync.dma_start(dl2t[:], dl2[:])
            dhdt = cp.tile([128, DHD_W], BF16, tag="dhdt")
            nc.sync.dma_start(dhdt[:], dhd[:])

            dvmt = cp.tile([128, DVM_W], BF16, tag="dvmt")
            nc.gpsimd.dma_start(dvmt[:], dvm[:])
            cft = cp.tile([128, CF_W], F32, tag="cft")
            nc.gpsimd.dma_start(cft[:], cf[:])

            # prefetch the gelu activation table during the const DMAs
            warm = wp.tile([1, 8], BF16, tag="warm")
            nc.scalar.activation(warm[:], Mt4[0:1, 0:8], AF.Gelu_apprx_tanh)

            ident = didt[:, 0:128]
            liftT = d0t[0:12, 0:128]
            liftin = d0t[0:12, 128:640]
            tmpN = [dt0t[:, 0:512], dl1t[:, 0:512], dl2t[:, 0:512]]
            EsT = [de0t[:, 0:512], dl1t[:, 512:1024], dl2t[:, 512:1024]]
            VTs = dy1t[:, 0:512]
            pwW = [dy1t[:, 640:768], dl1t[:, 1152:1280], dl2t[:, 1152:1280]]
            B4M = [dy1t[:, 512:640], dl1t[:, 1024:1152], dl2t[:, 1024:1152]]
            Vm = dvmt[:, 0:512]
            p1W = dhdt[:, 0:128]
            p2W = dhdt[:, 128:256]
            sel3 = dhdt[:, 256:260]
            pwb = [cft[:, 0:1], cft[:, 1:2]]
            p1b = cft[:, 2:3]
            p2b = cft[:, 3:4]
            b3c = cft[0:4, 4:5]

            # ---------------- lift ----------------
            liftp = pmix.tile([128, 512], F32, tag="mix")
            nc.tensor.matmul(liftp[:], liftT, liftin, start=True, stop=True)
            fq = wp.tile([128, 512], BF16, tag="fq")
            for h in range(2):
                nc.scalar.activation(
                    fq[:, H * h:H * (h + 1)], liftp[:, H * h:H * (h + 1)],
                    AF.Gelu_apprx_tanh,
                )

            # 4x-replicated moment basis, built on-device by a stride-0
            # broadcast copy as soon as Vm lands (DVE is idle then)
            Vm4 = cp.tile([128, 2048], BF16, tag="Vm4")
            nc.vector.tensor_copy(
                Vm4[:].rearrange("p (t r k) -> p t r k", t=16, r=4),
                Vm[:].rearrange("p (t k) -> p t k", t=16)
                .unsqueeze(2).broadcast_to((128, 16, 4, K)),
            )

            # ---------------- KNO layers ----------------
            for i in range(DEPTH):
                # natural layout: trp[p, 128m + 32j + c] = fq[512j+128m+p, c]
                trp = ptr.tile([128, 512], BF16, tag="trp")
                skp = pmix.tile([128, 512], F32, tag="mix")
                U = wp.tile([128, 512], BF16, tag="U")
                skps = wp.tile([128, 512], BF16, tag="skps")
                for m in range(4):
                    nc.tensor.transpose(
                        trp[:, 128 * m:128 * (m + 1)],
                        fq[:, 128 * m:128 * (m + 1)],
                        ident,
                    )
                # U = fq_nat * (w e^{-a x^2}), halves so moments start early
                for h in range(2):
                    nc.vector.tensor_mul(
                        U[:, H * h:H * (h + 1)],
                        trp[:, H * h:H * (h + 1)],
                        tmpN[i][:, H * h:H * (h + 1)],
                    )
                # replicated basis -> M lands on all 4 partition groups
                Mp4 = pmom.tile([128, C], F32, tag="Mp4", name=f"Mp4_{i}")
                for t in range(16):
                    m, j = divmod(t, 4)
                    nc.tensor.matmul(
                        Mp4[:],
                        Vm4[:, 128 * t:128 * (t + 1)],
                        U[:, 128 * m + 32 * j:128 * m + 32 * j + 32],
                        start=(t == 0),
                        stop=(t == 15),
                    )
                # skip^T via block-diagonal weights in the PE idle slot here
                # (bias rides the gelu / is folded into p1b for layer 2)
                for h in range(2):
                    nc.tensor.matmul(
                        skp[:, H * h:H * (h + 1)], pwW[i],
                        fq[:, H * h:H * (h + 1)], start=True, stop=True,
                    )
                    nc.scalar.copy(
                        skps[:, H * h:H * (h + 1)], skp[:, H * h:H * (h + 1)]
                    )
                nc.vector.tensor_mul(
                    Mt4[:].rearrange("p (r c) -> p r c", r=4),
                    Mp4[:].unsqueeze(1).broadcast_to((128, 4, C)),
                    B4M[i][:].rearrange("p (r c) -> p r c", r=4),
                )
                # eval + combine, pipelined in column halves:
                # fq_next = gelu(skip + pw_b + Es * (Mt4^T @ VTs))
                PT = pev.tile([128, 512], F32, tag="PT")
                z = wp.tile([128, 512], BF16, tag="z")
                pre = wp.tile([128, 512], BF16, tag="fq")
                if i < DEPTH - 1:
                    fq2 = wp.tile([128, 512], BF16, tag="fq", name=f"fq2_{i}")
                    csplit = [(0, 256), (256, 512)]
                else:
                    fq2 = pre
                    csplit = HSPLIT  # align with the head's asymmetric halves
                for lo, hi in csplit:
                    nc.tensor.matmul(
                        PT[:, lo:hi], Mt4[:], VTs[:, lo:hi], start=True, stop=True
                    )
                for lo, hi in csplit:
                    sl = slice(lo, hi)
                    nc.vector.tensor_mul(z[:, sl], PT[:, sl], EsT[i][:, sl])
                    nc.vector.tensor_add(pre[:, sl], z[:, sl], skps[:, sl])
                    if i < DEPTH - 1:
                        nc.scalar.activation(
                            fq2[:, sl], pre[:, sl], AF.Gelu_apprx_tanh,
                            bias=pwb[i],
                        )
                fq = fq2  # layer 2: pw_b[2] is folded into p1b on the host

            # ---------------- projection head (half-pipelined) ----------------
            pp1 = pmix.tile([128, 512], F32, tag="mix")
            g1 = wp.tile([128, 512], BF16, tag="fq")
            pp2 = pmix.tile([128, 512], F32, tag="mix")
            g2 = wp.tile([128, 512], BF16, tag="fq")
            pout = pop.tile([4, 512], F32, tag="pout")
            outsb = wp.tile([4, 512], F32, tag="outsb")
            for lo, hi in HSPLIT:
                sl = slice(lo, hi)
                nc.tensor.matmul(pp1[:, sl], p1W, fq[:, sl], start=True, stop=True)
                nc.scalar.activation(g1[:, sl], pp1[:, sl], AF.Gelu_apprx_tanh, bias=p1b)
                nc.tensor.matmul(pp2[:, sl], p2W, g1[:, sl], start=True, stop=True)
                nc.scalar.activation(g2[:, sl], pp2[:, sl], AF.Gelu_apprx_tanh, bias=p2b)
                # final dot: proj3_W folded into a selection stationary
                nc.tensor.matmul(pout[:, sl], sel3, g2[:, sl], start=True, stop=True)
                nc.vector.tensor_scalar_add(outsb[:, sl], pout[:, sl], b3c)
                eng = nc.gpsimd if lo == 0 else nc.sync
                eng.dma_start(out_dram[:, sl], outsb[:, sl])

    return nc


def get_nc():
    if "nc" not in _CACHE:
        nc = bacc.Bacc("TRN2", target_bir_lowering=False, debug=False, num_devices=NCORES)
        build_program(nc)
        nc.compile()
        _CACHE["nc"] = nc
    return _CACHE["nc"]


def make_in_map(
    f_x, x_grid, q_weights, lift_W, lift_b, pw_W, pw_b, ker_log_ell, ker_log_sigma,
    proj1_W, proj1_b, proj2_W, proj2_b, proj3_W, proj3_b,
):
    f8 = lambda a: np.asarray(a, dtype=np.float64)
    x = f8(x_grid).reshape(N)
    w = f8(q_weights).reshape(N)
    f = f8(f_x).reshape(N)
    a = 0.5 * np.exp(-2.0 * f8(ker_log_ell))          # [DEPTH, C]
    sig2 = np.exp(2.0 * f8(ker_log_sigma))            # [DEPTH, C]
    ks = np.arange(K, dtype=np.float64)
    lnfact = np.concatenate([[0.0], np.cumsum(np.log(np.arange(1, K)))])

    p = np.arange(128)
    npr = np.arange(512)

    did = np.eye(128)
    d0 = np.zeros((12, D0_W), np.float64)
    for j in range(4):
        d0[3 * j:3 * j + 2, 32 * j:32 * (j + 1)] = f8(lift_W).T
        d0[3 * j + 2, 32 * j:32 * (j + 1)] = f8(lift_b)
        nn = 512 * j + npr
        d0[3 * j, 128:640] = f[nn]
        d0[3 * j + 1, 128:640] = x[nn]
        d0[3 * j + 2, 128:640] = 1.0

    def bd(W):  # block-diag lhsT: [32j+c', 32j+c] = W[c, c']
        M = np.zeros((128, 128), np.float64)
        for j in range(4):
            M[32 * j:32 * (j + 1), 32 * j:32 * (j + 1)] = f8(W).T
        return M

    def tmpN_of(i):
        t = np.zeros((128, 512), np.float64)
        for m in range(4):
            for j in range(4):
                q = 512 * j + 128 * m + p
                t[:, 128 * m + 32 * j:128 * m + 32 * j + 32] = (
                    w[q, None] * np.exp(-a[i][None, :] * (x[q, None] ** 2)))
        return t

    def EsT_of(i):
        e = np.zeros((128, 512), np.float64)
        for j in range(4):
            nn = 512 * j + npr
            e[32 * j:32 * (j + 1), :] = (
                sig2[i][:, None] * np.exp(-a[i][:, None] * (x[None, nn] ** 2)))
        return e

    def B_of(i):  # B[k,c] = (2 a_c)^k / k!
        return np.exp(ks[:, None] * np.log(2.0 * a[i][None, :]) - lnfact[:, None])

    def B4M_of(i):
        M = np.zeros((128, 128), np.float64)
        B = B_of(i)
        for j in range(4):
            M[32 * j:32 * (j + 1), 32 * j:32 * (j + 1)] = B
        return M

    dt0 = tmpN_of(0)
    de0 = EsT_of(0)

    dhd = np.zeros((128, DHD_W), np.float64)
    dhd[:, 0:128] = bd(proj1_W)
    dhd[:, 128:256] = bd(proj2_W)
    for j in range(4):
        dhd[32 * j:32 * (j + 1), 256 + j] = f8(proj3_W)[0]

    dy1 = np.zeros((128, DY1_W), np.float64)
    for j in range(4):
        dy1[K * j:K * (j + 1), 0:512] = (
            x[None, 512 * j:512 * (j + 1)] ** ks[:, None])
    dy1[:, 512:640] = B4M_of(0)
    dy1[:, 640:768] = bd(pw_W[0])

    def dl_of(i):
        dl = np.zeros((128, DL_W), np.float64)
        dl[:, 0:512] = tmpN_of(i)
        dl[:, 512:1024] = EsT_of(i)
        dl[:, 1024:1152] = B4M_of(i)
        dl[:, 1152:1280] = bd(pw_W[i])
        return dl

    dvm = np.zeros((128, DVM_W), np.float64)
    for m in range(4):
        for j in range(4):
            q = 512 * j + 128 * m + p
            dvm[:, K * (4 * m + j):K * (4 * m + j + 1)] = x[q, None] ** ks[None, :]

    cfa = np.zeros((128, CF_W), np.float64)
    cfa[:, 0] = np.tile(f8(pw_b)[0], 4)
    cfa[:, 1] = np.tile(f8(pw_b)[1], 4)
    cfa[:, 2] = np.tile(f8(proj1_b) + f8(proj1_W) @ f8(pw_b)[2], 4)
    cfa[:, 3] = np.tile(f8(proj2_b), 4)
    cfa[0:4, 4] = f8(proj3_b)[0]

    return {
        "did": did.astype(NPBF16),
        "d0": d0.astype(NPBF16),
        "dt0": dt0.astype(NPBF16),
        "de0": de0.astype(NPBF16),
        "dhd": dhd.astype(NPBF16),
        "dy1": dy1.astype(NPBF16),
        "dl1": dl_of(1).astype(NPBF16),
        "dl2": dl_of(2).astype(NPBF16),
        "dvm": dvm.astype(NPBF16),
        "cf": cfa.astype(np.float32),
    }


def kernel(**inputs) -> np.ndarray:
    nc = get_nc()
    in_map = make_in_map(**inputs)
    res = run_bass_kernel_spmd(nc, [in_map] * NCORES, list(range(NCORES)))
    return np.asarray(res.results[0]["out"], dtype=np.float32).reshape(N)
